# revision 5
# baseline (speedup 1.0000x reference)
"""Trainium2 Bass kernel for MoE-LoRA fused QKV projection.

Computes, for x[32,512,1024], weight[3072,1024], per-sample top-2 LoRA
expert pools (rank 16) and expert biases:

    qkv = x @ W.T + bias
    qkv[..., :1024]  += SCALE * sum_k attn[b,k] * (x @ A_q[idx]) @ B_q[idx]
    qkv[..., 2048:]  += SCALE * sum_k attn[b,k] * (x @ A_v[idx]) @ B_v[idx]
    qkv += SCALE * sum_k attn[b,k] * bias_pool[idx]

Strategy: data-parallel over batch on 8 NeuronCores (4 samples/core).
Host side: transpose x / weight, gather + scale the tiny LoRA pools per
sample, fold all bias terms into one per-sample row vector that rides the
LoRA matmul via an appended ones-row (rank-1 matmul trick). Device side:
everything is dense fp32r (TF32-like, full PE rate) matmuls accumulated
in PSUM.
"""

import sys

for _p in ("/opt/trn_rl_repo",):
    if _p not in sys.path:
        sys.path.append(_p)

from contextlib import ExitStack

import numpy as np

import concourse.bass as bass
import concourse.tile as tile
from concourse import bacc, mybir
from concourse.bass_utils import run_bass_kernel_spmd

DIM = 1024
RANK = 16
TOPK = 2
BSZ = 32
SEQ = 512
SCALE = 1.0
NCORES = 8
SPC = BSZ // NCORES  # samples per core
NT = SEQ // 128  # token tiles per sample
KT = DIM // 128  # contraction tiles
NC3 = 3 * DIM
NCH = NC3 // 512  # output column chunks

F32 = mybir.dt.float32
F32R = mybir.dt.float32r

_NC_CACHE = {}


def _build_nc():
    nc = bacc.Bacc("TRN2", target_bir_lowering=False, debug=False)
    xT = nc.dram_tensor("xT", [SPC, DIM, SEQ], F32R, kind="ExternalInput").ap()
    wT = nc.dram_tensor("wT", [DIM, NC3], F32R, kind="ExternalInput").ap()
    acat = nc.dram_tensor("acat", [SPC, DIM, 4 * RANK], F32R, kind="ExternalInput").ap()
    bq = nc.dram_tensor("bq", [SPC, 33, DIM], F32R, kind="ExternalInput").ap()
    bv = nc.dram_tensor("bv", [SPC, 33, DIM], F32R, kind="ExternalInput").ap()
    bk = nc.dram_tensor("bk", [SPC, 1, DIM], F32R, kind="ExternalInput").ap()
    ones = nc.dram_tensor("ones", [2, SEQ], F32R, kind="ExternalInput").ap()
    out = nc.dram_tensor("out", [SPC, SEQ, NC3], F32, kind="ExternalOutput").ap()

    with tile.TileContext(nc) as tc, ExitStack() as ctx:
        wpool = ctx.enter_context(tc.tile_pool(name="w", bufs=1))
        xpool = ctx.enter_context(tc.tile_pool(name="x", bufs=2))
        apool = ctx.enter_context(tc.tile_pool(name="a", bufs=2))
        bpool = ctx.enter_context(tc.tile_pool(name="b", bufs=2))
        tpool = ctx.enter_context(tc.tile_pool(name="t", bufs=2))
        opool = ctx.enter_context(tc.tile_pool(name="o", bufs=2))
        pst = ctx.enter_context(tc.tile_pool(name="pst", bufs=2, space="PSUM"))
        pso = ctx.enter_context(tc.tile_pool(name="pso", bufs=3, space="PSUM"))

        def load_sample(s):
            x_sb = []
            for k in range(KT):
                t = xpool.tile([128, SEQ], F32R, tag=f"x{k}")
                nc.sync.dma_start(t[:], xT[s, k * 128 : (k + 1) * 128, :])
                x_sb.append(t)
            a_sb = apool.tile([128, KT * 4 * RANK], F32R, tag="a")
            for k in range(KT):
                nc.sync.dma_start(
                    a_sb[:, k * 64 : (k + 1) * 64],
                    acat[s, k * 128 : (k + 1) * 128, :],
                )
            bq_sb = bpool.tile([33, DIM], F32R, tag="bq")
            nc.sync.dma_start(bq_sb[:], bq[s])
            bv_sb = bpool.tile([33, DIM], F32R, tag="bv")
            nc.sync.dma_start(bv_sb[:], bv[s])
            bk_sb = bpool.tile([1, DIM], F32R, tag="bk")
            nc.sync.dma_start(bk_sb[:], bk[s])
            return x_sb, a_sb, bq_sb, bv_sb, bk_sb

        loaded = {0: load_sample(0)}

        # Resident transposed base weight: 48 tiles [128, 512], chunk-major
        # so the first output chunk's tiles land before the rest.
        w_sb = {}
        for c in range(NCH):
            for k in range(KT):
                t = wpool.tile([128, 512], F32R, tag=f"w{k}_{c}")
                nc.sync.dma_start(
                    t[:], wT[k * 128 : (k + 1) * 128, c * 512 : (c + 1) * 512]
                )
                w_sb[(k, c)] = t

        for s in range(SPC):
            if s not in loaded:
                loaded[s] = load_sample(s)
            x_sb, a_sb, bq_sb, bv_sb, bk_sb = loaded.pop(s)

            # t_q[32, 512] = A_q_cat.T @ x ; t_v likewise (separate PSUM
            # tiles so SBUF copies stay partition-aligned)
            pt_q = pst.tile([32, SEQ], F32, tag="ptq")
            pt_v = pst.tile([32, SEQ], F32, tag="ptv")
            for k in range(KT):
                nc.tensor.matmul(
                    pt_q[:],
                    a_sb[:, k * 64 : k * 64 + 32],
                    x_sb[k][:],
                    start=(k == 0),
                    stop=(k == KT - 1),
                )
            for k in range(KT):
                nc.tensor.matmul(
                    pt_v[:],
                    a_sb[:, k * 64 + 32 : (k + 1) * 64],
                    x_sb[k][:],
                    start=(k == 0),
                    stop=(k == KT - 1),
                )
            # tq/tv rows 0-31 = t, row 32 = ones (bias rides rank-1 row)
            tq_sb = tpool.tile([33, SEQ], F32R, tag="tq")
            tv_sb = tpool.tile([33, SEQ], F32R, tag="tv")
            ones_sb = tpool.tile([1, SEQ], F32R, tag="ones1")
            nc.sync.dma_start(tq_sb[32:33, :], ones[0:1, :])
            nc.sync.dma_start(tv_sb[32:33, :], ones[0:1, :])
            nc.sync.dma_start(ones_sb[:], ones[1:2, :])
            nc.vector.tensor_copy(tq_sb[0:32, :], pt_q[:])
            nc.vector.tensor_copy(tv_sb[0:32, :], pt_v[:])

            for nt in range(NT):
                o_sb = opool.tile([128, NC3], F32, tag="o")
                tok = bass.ts(nt, 128)
                for c in range(NCH):
                    po = pso.tile([128, 512], F32, tag="po")
                    for k in range(KT):
                        nc.tensor.matmul(
                            po[:],
                            x_sb[k][:, tok],
                            w_sb[(k, c)][:],
                            start=(k == 0),
                            stop=False,
                        )
                    if c < 2:
                        nc.tensor.matmul(
                            po[:],
                            tq_sb[:, tok],
                            bq_sb[:, bass.ts(c, 512)],
                            start=False,
                            stop=True,
                        )
                    elif c >= 4:
                        nc.tensor.matmul(
                            po[:],
                            tv_sb[:, tok],
                            bv_sb[:, bass.ts(c - 4, 512)],
                            start=False,
                            stop=True,
                        )
                    else:
                        nc.tensor.matmul(
                            po[:],
                            ones_sb[:, tok],
                            bk_sb[:, bass.ts(c - 2, 512)],
                            start=False,
                            stop=True,
                        )
                    if c % 2 == 0:
                        nc.vector.tensor_copy(o_sb[:, bass.ts(c, 512)], po[:])
                    else:
                        nc.scalar.copy(o_sb[:, bass.ts(c, 512)], po[:])
                nc.sync.dma_start(out[s, tok, :], o_sb[:])

    nc.compile()
    return nc


def _get_nc():
    if "nc" not in _NC_CACHE:
        _NC_CACHE["nc"] = _build_nc()
    return _NC_CACHE["nc"]


def kernel(**inputs):
    x = np.asarray(inputs["x"], dtype=np.float32)
    weight = np.asarray(inputs["weight"], dtype=np.float32)
    bias = np.asarray(inputs["bias"], dtype=np.float32)
    A_q = np.asarray(inputs["A_q_pool"], dtype=np.float32)
    B_q = np.asarray(inputs["B_q_pool"], dtype=np.float32)
    A_v = np.asarray(inputs["A_v_pool"], dtype=np.float32)
    B_v = np.asarray(inputs["B_v_pool"], dtype=np.float32)
    bias_pool = np.asarray(inputs["bias_pool"], dtype=np.float32)
    attn = np.asarray(inputs["attn"], dtype=np.float32)
    idx = np.asarray(inputs["idx"]).astype(np.int64)

    # -- host-side prep (cheap; pools are tiny) --
    xT = np.ascontiguousarray(x.transpose(0, 2, 1))  # [B, DIM, SEQ]
    wT = np.ascontiguousarray(weight.T)  # [DIM, 3*DIM]

    i0, i1 = idx[:, 0], idx[:, 1]
    # [B, DIM, 64]: columns 0-15 q/k0, 16-31 q/k1, 32-47 v/k0, 48-63 v/k1
    acat = np.concatenate([A_q[i0], A_q[i1], A_v[i0], A_v[i1]], axis=2)
    acat = np.ascontiguousarray(acat)

    wgt = (SCALE * attn)[:, :, None, None]  # [B, K, 1, 1]
    bq_s = (B_q[idx] * wgt).reshape(BSZ, TOPK * RANK, DIM)
    bv_s = (B_v[idx] * wgt).reshape(BSZ, TOPK * RANK, DIM)
    bias_total = bias[None, :] + SCALE * np.einsum(
        "bko,bk->bo", bias_pool[idx], attn
    )  # [B, 3*DIM]
    bq_ext = np.ascontiguousarray(
        np.concatenate([bq_s, bias_total[:, None, :DIM]], axis=1)
    )  # [B, 33, DIM]
    bv_ext = np.ascontiguousarray(
        np.concatenate([bv_s, bias_total[:, None, 2 * DIM :]], axis=1)
    )  # [B, 33, DIM]
    bk_row = np.ascontiguousarray(bias_total[:, None, DIM : 2 * DIM])  # [B, 1, DIM]
    ones = np.ones((2, SEQ), dtype=np.float32)

    nc = _get_nc()
    in_maps = []
    for c in range(NCORES):
        sl = slice(c * SPC, (c + 1) * SPC)
        in_maps.append(
            {
                "xT": xT[sl],
                "wT": wT,
                "acat": acat[sl],
                "bq": bq_ext[sl],
                "bv": bv_ext[sl],
                "bk": bk_row[sl],
                "ones": ones,
            }
        )
    global _LAST_IN_MAPS
    _LAST_IN_MAPS = in_maps
    res = run_bass_kernel_spmd(nc, in_maps, list(range(NCORES)))
    out = np.concatenate(
        [np.asarray(res.results[i]["out"]) for i in range(NCORES)], axis=0
    )
    return out.astype(np.float32, copy=False)


_LAST_IN_MAPS = None


# revision 9
# speedup vs baseline: 1.0272x; 1.0272x over previous
"""Trainium2 Bass kernel for MoE-LoRA fused QKV projection.

Computes, for x[32,512,1024], weight[3072,1024], per-sample top-2 LoRA
expert pools (rank 16) and expert biases:

    qkv = x @ W.T + bias
    qkv[..., :1024]  += SCALE * sum_k attn[b,k] * (x @ A_q[idx]) @ B_q[idx]
    qkv[..., 2048:]  += SCALE * sum_k attn[b,k] * (x @ A_v[idx]) @ B_v[idx]
    qkv += SCALE * sum_k attn[b,k] * bias_pool[idx]

Strategy: data-parallel over batch on 8 NeuronCores (4 samples/core).
Host side: transpose x / weight, gather + scale the tiny LoRA pools per
sample, fold all bias terms into one per-sample row vector that rides the
LoRA matmul via an appended ones-row (rank-1 matmul trick). Device side:
everything is dense fp32r (TF32-like, full PE rate) matmuls accumulated
in PSUM.
"""

import sys

for _p in ("/opt/trn_rl_repo",):
    if _p not in sys.path:
        sys.path.append(_p)

from contextlib import ExitStack

import numpy as np

import concourse.bass as bass
import concourse.tile as tile
from concourse import bacc, mybir
from concourse.bass_utils import run_bass_kernel_spmd

DIM = 1024
RANK = 16
TOPK = 2
BSZ = 32
SEQ = 512
SCALE = 1.0
NCORES = 8
SPC = BSZ // NCORES  # samples per core
NT = SEQ // 128  # token tiles per sample
KT = DIM // 128  # contraction tiles
NC3 = 3 * DIM
NCH = NC3 // 512  # output column chunks

F32 = mybir.dt.float32
F32R = mybir.dt.float32r

_NC_CACHE = {}


def _build_nc():
    nc = bacc.Bacc("TRN2", target_bir_lowering=False, debug=False)
    xT = nc.dram_tensor("xT", [SPC, DIM, SEQ], F32R, kind="ExternalInput").ap()
    wT = nc.dram_tensor("wT", [DIM, NC3], F32R, kind="ExternalInput").ap()
    acat = nc.dram_tensor("acat", [SPC, DIM, 4 * RANK], F32R, kind="ExternalInput").ap()
    bq = nc.dram_tensor("bq", [SPC, 33, DIM], F32R, kind="ExternalInput").ap()
    bv = nc.dram_tensor("bv", [SPC, 33, DIM], F32R, kind="ExternalInput").ap()
    bk = nc.dram_tensor("bk", [SPC, 1, DIM], F32R, kind="ExternalInput").ap()
    ones = nc.dram_tensor("ones", [2, SEQ], F32R, kind="ExternalInput").ap()
    out = nc.dram_tensor("out", [SPC, SEQ, NC3], F32, kind="ExternalOutput").ap()

    with tile.TileContext(nc) as tc, ExitStack() as ctx:
        wpool = ctx.enter_context(tc.tile_pool(name="w", bufs=1))
        xpool = ctx.enter_context(tc.tile_pool(name="x", bufs=2))
        apool = ctx.enter_context(tc.tile_pool(name="a", bufs=2))
        bpool = ctx.enter_context(tc.tile_pool(name="b", bufs=2))
        tpool = ctx.enter_context(tc.tile_pool(name="t", bufs=2))
        opool = ctx.enter_context(tc.tile_pool(name="o", bufs=4))
        pst = ctx.enter_context(tc.tile_pool(name="pst", bufs=2, space="PSUM"))
        pso = ctx.enter_context(tc.tile_pool(name="pso", bufs=3, space="PSUM"))

        def load_sample(s):
            x_sb = []
            a_sb = []
            for k in range(KT):
                ta = apool.tile([128, 4 * RANK], F32R, tag=f"a{k}")
                nc.sync.dma_start(ta[:], acat[s, k * 128 : (k + 1) * 128, :])
                a_sb.append(ta)
                t = xpool.tile([128, SEQ], F32R, tag=f"x{k}")
                nc.sync.dma_start(t[:], xT[s, k * 128 : (k + 1) * 128, :])
                x_sb.append(t)
            bq_sb = bpool.tile([33, DIM], F32R, tag="bq")
            nc.sync.dma_start(bq_sb[:], bq[s])
            bv_sb = bpool.tile([33, DIM], F32R, tag="bv")
            nc.sync.dma_start(bv_sb[:], bv[s])
            bk_sb = bpool.tile([1, DIM], F32R, tag="bk")
            nc.sync.dma_start(bk_sb[:], bk[s])
            return x_sb, a_sb, bq_sb, bv_sb, bk_sb

        loaded = {0: load_sample(0)}

        # Resident transposed base weight: 48 tiles [128, 512], chunk-major
        # so the first output chunk's tiles land before the rest.
        w_sb = {}
        for c in range(NCH):
            for k in range(KT):
                t = wpool.tile([128, 512], F32R, tag=f"w{k}_{c}")
                nc.sync.dma_start(
                    t[:], wT[k * 128 : (k + 1) * 128, c * 512 : (c + 1) * 512]
                )
                w_sb[(k, c)] = t

        for s in range(SPC):
            if s not in loaded:
                loaded[s] = load_sample(s)
            x_sb, a_sb, bq_sb, bv_sb, bk_sb = loaded.pop(s)

            # t_q[32, 512] = A_q_cat.T @ x ; t_v likewise (separate PSUM
            # tiles so SBUF copies stay partition-aligned)
            pt_q = pst.tile([32, SEQ], F32, tag="ptq")
            pt_v = pst.tile([32, SEQ], F32, tag="ptv")
            for k in range(KT):
                nc.tensor.matmul(
                    pt_q[:],
                    a_sb[k][:, 0:32],
                    x_sb[k][:],
                    start=(k == 0),
                    stop=(k == KT - 1),
                )
            for k in range(KT):
                nc.tensor.matmul(
                    pt_v[:],
                    a_sb[k][:, 32:64],
                    x_sb[k][:],
                    start=(k == 0),
                    stop=(k == KT - 1),
                )
            # tq/tv rows 0-31 = t, row 32 = ones (bias rides rank-1 row)
            tq_sb = tpool.tile([33, SEQ], F32R, tag="tq")
            tv_sb = tpool.tile([33, SEQ], F32R, tag="tv")
            ones_sb = tpool.tile([1, SEQ], F32R, tag="ones1")
            nc.sync.dma_start(tq_sb[32:33, :], ones[0:1, :])
            nc.sync.dma_start(tv_sb[32:33, :], ones[0:1, :])
            nc.sync.dma_start(ones_sb[:], ones[1:2, :])
            nc.vector.tensor_copy(tq_sb[0:32, :], pt_q[:])
            nc.vector.tensor_copy(tv_sb[0:32, :], pt_v[:])

            # chunk-outer so the base-weight stream is consumed at DMA pace
            for c in range(NCH):
                for nt in range(NT):
                    tok = bass.ts(nt, 128)
                    po = pso.tile([128, 512], F32, tag="po")
                    for k in range(KT):
                        nc.tensor.matmul(
                            po[:],
                            x_sb[k][:, tok],
                            w_sb[(k, c)][:],
                            start=(k == 0),
                            stop=False,
                        )
                    if c < 2:
                        nc.tensor.matmul(
                            po[:],
                            tq_sb[:, tok],
                            bq_sb[:, bass.ts(c, 512)],
                            start=False,
                            stop=True,
                        )
                    elif c >= 4:
                        nc.tensor.matmul(
                            po[:],
                            tv_sb[:, tok],
                            bv_sb[:, bass.ts(c - 4, 512)],
                            start=False,
                            stop=True,
                        )
                    else:
                        nc.tensor.matmul(
                            po[:],
                            ones_sb[:, tok],
                            bk_sb[:, bass.ts(c - 2, 512)],
                            start=False,
                            stop=True,
                        )
                    o_sb = opool.tile([128, 512], F32, tag="o")
                    if (c * NT + nt) % 2 == 0:
                        nc.vector.tensor_copy(o_sb[:], po[:])
                    else:
                        nc.scalar.copy(o_sb[:], po[:])
                    nc.scalar.dma_start(
                        out[s, tok, bass.ts(c, 512)], o_sb[:]
                    )

    nc.compile()
    return nc


def _get_nc():
    if "nc" not in _NC_CACHE:
        _NC_CACHE["nc"] = _build_nc()
    return _NC_CACHE["nc"]


def kernel(**inputs):
    x = np.asarray(inputs["x"], dtype=np.float32)
    weight = np.asarray(inputs["weight"], dtype=np.float32)
    bias = np.asarray(inputs["bias"], dtype=np.float32)
    A_q = np.asarray(inputs["A_q_pool"], dtype=np.float32)
    B_q = np.asarray(inputs["B_q_pool"], dtype=np.float32)
    A_v = np.asarray(inputs["A_v_pool"], dtype=np.float32)
    B_v = np.asarray(inputs["B_v_pool"], dtype=np.float32)
    bias_pool = np.asarray(inputs["bias_pool"], dtype=np.float32)
    attn = np.asarray(inputs["attn"], dtype=np.float32)
    idx = np.asarray(inputs["idx"]).astype(np.int64)

    # -- host-side prep (cheap; pools are tiny) --
    xT = np.ascontiguousarray(x.transpose(0, 2, 1))  # [B, DIM, SEQ]
    wT = np.ascontiguousarray(weight.T)  # [DIM, 3*DIM]

    i0, i1 = idx[:, 0], idx[:, 1]
    # [B, DIM, 64]: columns 0-15 q/k0, 16-31 q/k1, 32-47 v/k0, 48-63 v/k1
    acat = np.concatenate([A_q[i0], A_q[i1], A_v[i0], A_v[i1]], axis=2)
    acat = np.ascontiguousarray(acat)

    wgt = (SCALE * attn)[:, :, None, None]  # [B, K, 1, 1]
    bq_s = (B_q[idx] * wgt).reshape(BSZ, TOPK * RANK, DIM)
    bv_s = (B_v[idx] * wgt).reshape(BSZ, TOPK * RANK, DIM)
    bias_total = bias[None, :] + SCALE * np.einsum(
        "bko,bk->bo", bias_pool[idx], attn
    )  # [B, 3*DIM]
    bq_ext = np.ascontiguousarray(
        np.concatenate([bq_s, bias_total[:, None, :DIM]], axis=1)
    )  # [B, 33, DIM]
    bv_ext = np.ascontiguousarray(
        np.concatenate([bv_s, bias_total[:, None, 2 * DIM :]], axis=1)
    )  # [B, 33, DIM]
    bk_row = np.ascontiguousarray(bias_total[:, None, DIM : 2 * DIM])  # [B, 1, DIM]
    ones = np.ones((2, SEQ), dtype=np.float32)

    nc = _get_nc()
    in_maps = []
    for c in range(NCORES):
        sl = slice(c * SPC, (c + 1) * SPC)
        in_maps.append(
            {
                "xT": xT[sl],
                "wT": wT,
                "acat": acat[sl],
                "bq": bq_ext[sl],
                "bv": bv_ext[sl],
                "bk": bk_row[sl],
                "ones": ones,
            }
        )
    global _LAST_IN_MAPS
    _LAST_IN_MAPS = in_maps
    res = run_bass_kernel_spmd(nc, in_maps, list(range(NCORES)))
    out = np.concatenate(
        [np.asarray(res.results[i]["out"]) for i in range(NCORES)], axis=0
    )
    return out.astype(np.float32, copy=False)


_LAST_IN_MAPS = None


# revision 12
# speedup vs baseline: 1.1174x; 1.0878x over previous
"""Trainium2 Bass kernel for MoE-LoRA fused QKV projection.

Computes, for x[32,512,1024], weight[3072,1024], per-sample top-2 LoRA
expert pools (rank 16) and expert biases:

    qkv = x @ W.T + bias
    qkv[..., :1024]  += SCALE * sum_k attn[b,k] * (x @ A_q[idx]) @ B_q[idx]
    qkv[..., 2048:]  += SCALE * sum_k attn[b,k] * (x @ A_v[idx]) @ B_v[idx]
    qkv += SCALE * sum_k attn[b,k] * bias_pool[idx]

Strategy: data-parallel over batch on 8 NeuronCores (4 samples/core).
Host side: transpose x / weight, gather + scale the tiny LoRA pools per
sample, fold all bias terms into one per-sample row vector that rides the
LoRA matmul via an appended ones-row (rank-1 matmul trick). Device side:
everything is dense fp32r (TF32-like, full PE rate) matmuls accumulated
in PSUM.
"""

import sys

for _p in ("/opt/trn_rl_repo",):
    if _p not in sys.path:
        sys.path.append(_p)

from contextlib import ExitStack

import numpy as np

import concourse.bass as bass
import concourse.tile as tile
from concourse import bacc, mybir
from concourse.bass_utils import run_bass_kernel_spmd

DIM = 1024
RANK = 16
TOPK = 2
BSZ = 32
SEQ = 512
SCALE = 1.0
NCORES = 8
SPC = BSZ // NCORES  # samples per core
NT = SEQ // 128  # token tiles per sample
KT = DIM // 128  # contraction tiles
NC3 = 3 * DIM
NCH = NC3 // 512  # output column chunks

F32 = mybir.dt.float32
F32R = mybir.dt.float32r

_NC_CACHE = {}


def _build_nc():
    nc = bacc.Bacc("TRN2", target_bir_lowering=False, debug=False)
    xT = nc.dram_tensor("xT", [SPC, DIM, SEQ], F32R, kind="ExternalInput").ap()
    wT = nc.dram_tensor("wT", [DIM, NC3], F32R, kind="ExternalInput").ap()
    acat = nc.dram_tensor("acat", [SPC, DIM, 4 * RANK], F32R, kind="ExternalInput").ap()
    bq = nc.dram_tensor("bq", [SPC, 33, DIM], F32R, kind="ExternalInput").ap()
    bv = nc.dram_tensor("bv", [SPC, 33, DIM], F32R, kind="ExternalInput").ap()
    bk = nc.dram_tensor("bk", [SPC, 1, DIM], F32R, kind="ExternalInput").ap()
    ones = nc.dram_tensor("ones", [2, SEQ], F32R, kind="ExternalInput").ap()
    out = nc.dram_tensor("out", [SPC, SEQ, NC3], F32, kind="ExternalOutput").ap()

    with tile.TileContext(nc) as tc, ExitStack() as ctx:
        wpool = ctx.enter_context(tc.tile_pool(name="w", bufs=1))
        xpool = ctx.enter_context(tc.tile_pool(name="x", bufs=2))
        apool = ctx.enter_context(tc.tile_pool(name="a", bufs=2))
        bpool = ctx.enter_context(tc.tile_pool(name="b", bufs=2))
        tpool = ctx.enter_context(tc.tile_pool(name="t", bufs=2))
        opool = ctx.enter_context(tc.tile_pool(name="o", bufs=3))
        pst = ctx.enter_context(tc.tile_pool(name="pst", bufs=2, space="PSUM"))
        pso = ctx.enter_context(tc.tile_pool(name="pso", bufs=3, space="PSUM"))

        def load_sample(s):
            # x: one 2 MB DMA [DIM, SEQ] -> [128, (k j)]
            x_t = xpool.tile([128, KT * SEQ], F32R, tag="x")
            nc.sync.dma_start(
                x_t[:].rearrange("p (k j) -> p k j", j=SEQ),
                xT[s].rearrange("(k p) j -> p k j", p=128),
            )
            # A_cat: one DMA [DIM, 64] -> [128, (k r)]
            a_t = apool.tile([128, KT * 4 * RANK], F32R, tag="a")
            nc.sync.dma_start(
                a_t[:].rearrange("p (k r) -> p k r", r=4 * RANK),
                acat[s].rearrange("(k p) r -> p k r", p=128),
            )
            bq_sb = bpool.tile([33, DIM], F32R, tag="bq")
            nc.scalar.dma_start(bq_sb[:], bq[s])
            bv_sb = bpool.tile([33, DIM], F32R, tag="bv")
            nc.scalar.dma_start(bv_sb[:], bv[s])
            bk_sb = bpool.tile([1, DIM], F32R, tag="bk")
            nc.scalar.dma_start(bk_sb[:], bk[s])
            return x_t, a_t, bq_sb, bv_sb, bk_sb

        loaded = {0: load_sample(0)}

        # Resident transposed base weight: 24 tiles [128, 1024] (512 KB
        # DMAs), chunk-pair-major so early chunks land first.
        w_sb = {}
        for cp in range(NCH // 2):
            for k in range(KT):
                t = wpool.tile([128, 1024], F32R, tag=f"w{k}_{cp}")
                nc.sync.dma_start(
                    t[:],
                    wT[k * 128 : (k + 1) * 128, cp * 1024 : (cp + 1) * 1024],
                )
                w_sb[(k, cp)] = t

        for s in range(SPC):
            if s not in loaded:
                loaded[s] = load_sample(s)
            x_t, a_t, bq_sb, bv_sb, bk_sb = loaded.pop(s)

            def xk(k):
                return x_t[:, bass.ts(k, SEQ)]

            # t_q[32, 512] = A_q_cat.T @ x ; t_v likewise (separate PSUM
            # tiles so SBUF copies stay partition-aligned)
            pt_q = pst.tile([32, SEQ], F32, tag="ptq")
            pt_v = pst.tile([32, SEQ], F32, tag="ptv")
            for k in range(KT):
                nc.tensor.matmul(
                    pt_q[:],
                    a_t[:, k * 64 : k * 64 + 32],
                    xk(k),
                    start=(k == 0),
                    stop=(k == KT - 1),
                )
            for k in range(KT):
                nc.tensor.matmul(
                    pt_v[:],
                    a_t[:, k * 64 + 32 : (k + 1) * 64],
                    xk(k),
                    start=(k == 0),
                    stop=(k == KT - 1),
                )
            # tq/tv rows 0-31 = t, row 32 = ones (bias rides rank-1 row)
            tq_sb = tpool.tile([33, SEQ], F32R, tag="tq")
            tv_sb = tpool.tile([33, SEQ], F32R, tag="tv")
            ones_sb = tpool.tile([1, SEQ], F32R, tag="ones1")
            nc.scalar.dma_start(tq_sb[32:33, :], ones[0:1, :])
            nc.scalar.dma_start(tv_sb[32:33, :], ones[0:1, :])
            nc.scalar.dma_start(ones_sb[:], ones[1:2, :])
            nc.vector.tensor_copy(tq_sb[0:32, :], pt_q[:])
            nc.vector.tensor_copy(tv_sb[0:32, :], pt_v[:])

            # chunk-pair-outer so the base-weight stream is consumed at
            # DMA pace; stores batched per [128, 1024] chunk-pair
            for cp in range(NCH // 2):
                for nt in range(NT):
                    tok = bass.ts(nt, 128)
                    o_sb = opool.tile([128, 1024], F32, tag="o")
                    for half in range(2):
                        c = cp * 2 + half
                        po = pso.tile([128, 512], F32, tag="po")
                        for k in range(KT):
                            nc.tensor.matmul(
                                po[:],
                                x_t[:, k * SEQ + nt * 128 : k * SEQ + (nt + 1) * 128],
                                w_sb[(k, cp)][:, bass.ts(half, 512)],
                                start=(k == 0),
                                stop=False,
                            )
                        if c < 2:
                            nc.tensor.matmul(
                                po[:],
                                tq_sb[:, tok],
                                bq_sb[:, bass.ts(c, 512)],
                                start=False,
                                stop=True,
                            )
                        elif c >= 4:
                            nc.tensor.matmul(
                                po[:],
                                tv_sb[:, tok],
                                bv_sb[:, bass.ts(c - 4, 512)],
                                start=False,
                                stop=True,
                            )
                        else:
                            nc.tensor.matmul(
                                po[:],
                                ones_sb[:, tok],
                                bk_sb[:, bass.ts(c - 2, 512)],
                                start=False,
                                stop=True,
                            )
                        if (c * NT + nt) % 2 == 0:
                            nc.vector.tensor_copy(
                                o_sb[:, bass.ts(half, 512)], po[:]
                            )
                        else:
                            nc.scalar.copy(o_sb[:, bass.ts(half, 512)], po[:])
                    nc.scalar.dma_start(
                        out[s, tok, bass.ts(cp, 1024)], o_sb[:]
                    )

    nc.compile()
    return nc


def _get_nc():
    if "nc" not in _NC_CACHE:
        _NC_CACHE["nc"] = _build_nc()
    return _NC_CACHE["nc"]


def kernel(**inputs):
    x = np.asarray(inputs["x"], dtype=np.float32)
    weight = np.asarray(inputs["weight"], dtype=np.float32)
    bias = np.asarray(inputs["bias"], dtype=np.float32)
    A_q = np.asarray(inputs["A_q_pool"], dtype=np.float32)
    B_q = np.asarray(inputs["B_q_pool"], dtype=np.float32)
    A_v = np.asarray(inputs["A_v_pool"], dtype=np.float32)
    B_v = np.asarray(inputs["B_v_pool"], dtype=np.float32)
    bias_pool = np.asarray(inputs["bias_pool"], dtype=np.float32)
    attn = np.asarray(inputs["attn"], dtype=np.float32)
    idx = np.asarray(inputs["idx"]).astype(np.int64)

    # -- host-side prep (cheap; pools are tiny) --
    xT = np.ascontiguousarray(x.transpose(0, 2, 1))  # [B, DIM, SEQ]
    wT = np.ascontiguousarray(weight.T)  # [DIM, 3*DIM]

    i0, i1 = idx[:, 0], idx[:, 1]
    # [B, DIM, 64]: columns 0-15 q/k0, 16-31 q/k1, 32-47 v/k0, 48-63 v/k1
    acat = np.concatenate([A_q[i0], A_q[i1], A_v[i0], A_v[i1]], axis=2)
    acat = np.ascontiguousarray(acat)

    wgt = (SCALE * attn)[:, :, None, None]  # [B, K, 1, 1]
    bq_s = (B_q[idx] * wgt).reshape(BSZ, TOPK * RANK, DIM)
    bv_s = (B_v[idx] * wgt).reshape(BSZ, TOPK * RANK, DIM)
    bias_total = bias[None, :] + SCALE * np.einsum(
        "bko,bk->bo", bias_pool[idx], attn
    )  # [B, 3*DIM]
    bq_ext = np.ascontiguousarray(
        np.concatenate([bq_s, bias_total[:, None, :DIM]], axis=1)
    )  # [B, 33, DIM]
    bv_ext = np.ascontiguousarray(
        np.concatenate([bv_s, bias_total[:, None, 2 * DIM :]], axis=1)
    )  # [B, 33, DIM]
    bk_row = np.ascontiguousarray(bias_total[:, None, DIM : 2 * DIM])  # [B, 1, DIM]
    ones = np.ones((2, SEQ), dtype=np.float32)

    nc = _get_nc()
    in_maps = []
    for c in range(NCORES):
        sl = slice(c * SPC, (c + 1) * SPC)
        in_maps.append(
            {
                "xT": xT[sl],
                "wT": wT,
                "acat": acat[sl],
                "bq": bq_ext[sl],
                "bv": bv_ext[sl],
                "bk": bk_row[sl],
                "ones": ones,
            }
        )
    global _LAST_IN_MAPS
    _LAST_IN_MAPS = in_maps
    res = run_bass_kernel_spmd(nc, in_maps, list(range(NCORES)))
    out = np.concatenate(
        [np.asarray(res.results[i]["out"]) for i in range(NCORES)], axis=0
    )
    return out.astype(np.float32, copy=False)


_LAST_IN_MAPS = None


# revision 16
# speedup vs baseline: 1.1214x; 1.0035x over previous
"""Trainium2 Bass kernel for MoE-LoRA fused QKV projection.

Computes, for x[32,512,1024], weight[3072,1024], per-sample top-2 LoRA
expert pools (rank 16) and expert biases:

    qkv = x @ W.T + bias
    qkv[..., :1024]  += SCALE * sum_k attn[b,k] * (x @ A_q[idx]) @ B_q[idx]
    qkv[..., 2048:]  += SCALE * sum_k attn[b,k] * (x @ A_v[idx]) @ B_v[idx]
    qkv += SCALE * sum_k attn[b,k] * bias_pool[idx]

Strategy: data-parallel over batch on 8 NeuronCores (4 samples/core).
Host side: transpose x / weight, gather + scale the tiny LoRA pools per
sample, fold all bias terms into one per-sample row vector that rides the
LoRA matmul via an appended ones-row (rank-1 matmul trick). Device side:
everything is dense fp32r (TF32-like, full PE rate) matmuls accumulated
in PSUM.
"""

import sys

for _p in ("/opt/trn_rl_repo",):
    if _p not in sys.path:
        sys.path.append(_p)

from contextlib import ExitStack

import numpy as np

import concourse.bass as bass
import concourse.tile as tile
from concourse import bacc, mybir
from concourse.bass_utils import run_bass_kernel_spmd

DIM = 1024
RANK = 16
TOPK = 2
BSZ = 32
SEQ = 512
SCALE = 1.0
NCORES = 8
SPC = BSZ // NCORES  # samples per core
NT = SEQ // 128  # token tiles per sample
KT = DIM // 128  # contraction tiles
NC3 = 3 * DIM
NCH = NC3 // 512  # output column chunks

F32 = mybir.dt.float32
F32R = mybir.dt.float32r

_NC_CACHE = {}


def _build_nc():
    nc = bacc.Bacc("TRN2", target_bir_lowering=False, debug=False)
    xT = nc.dram_tensor("xT", [SPC, DIM, SEQ], F32R, kind="ExternalInput").ap()
    wT = nc.dram_tensor("wT", [DIM, NC3], F32R, kind="ExternalInput").ap()
    acat = nc.dram_tensor("acat", [SPC, DIM, 4 * RANK], F32R, kind="ExternalInput").ap()
    bq = nc.dram_tensor("bq", [SPC, 33, DIM], F32R, kind="ExternalInput").ap()
    bv = nc.dram_tensor("bv", [SPC, 33, DIM], F32R, kind="ExternalInput").ap()
    bk = nc.dram_tensor("bk", [SPC, 1, DIM], F32R, kind="ExternalInput").ap()
    ones = nc.dram_tensor("ones", [2, SEQ], F32R, kind="ExternalInput").ap()
    out = nc.dram_tensor("out", [SPC, SEQ, NC3], F32, kind="ExternalOutput").ap()

    with tile.TileContext(nc) as tc, ExitStack() as ctx:
        wpool = ctx.enter_context(tc.tile_pool(name="w", bufs=1))
        xpool = ctx.enter_context(tc.tile_pool(name="x", bufs=2))
        apool = ctx.enter_context(tc.tile_pool(name="a", bufs=2))
        bpool = ctx.enter_context(tc.tile_pool(name="b", bufs=2))
        tpool = ctx.enter_context(tc.tile_pool(name="t", bufs=2))
        opool = ctx.enter_context(tc.tile_pool(name="o", bufs=3))
        pst = ctx.enter_context(tc.tile_pool(name="pst", bufs=1, space="PSUM"))
        pso = ctx.enter_context(tc.tile_pool(name="pso", bufs=5, space="PSUM"))

        def load_sample(s):
            # x: two 1 MB DMAs [DIM, SEQ] -> [128, (k j)] so the first
            # t-matmuls can start at the half-way mark
            x_t = xpool.tile([128, KT * SEQ], F32R, tag="x")
            h = KT // 2
            for i in range(2):
                nc.sync.dma_start(
                    x_t[:, i * h * SEQ : (i + 1) * h * SEQ].rearrange(
                        "p (k j) -> p k j", j=SEQ
                    ),
                    xT[s][i * h * 128 : (i + 1) * h * 128, :].rearrange(
                        "(k p) j -> p k j", p=128
                    ),
                )
            # A_cat: one DMA [DIM, 64] -> [128, (k r)]
            a_t = apool.tile([128, KT * 4 * RANK], F32R, tag="a")
            nc.sync.dma_start(
                a_t[:].rearrange("p (k r) -> p k r", r=4 * RANK),
                acat[s].rearrange("(k p) r -> p k r", p=128),
            )
            bq_sb = bpool.tile([33, DIM], F32R, tag="bq")
            nc.scalar.dma_start(bq_sb[:], bq[s])
            bv_sb = bpool.tile([33, DIM], F32R, tag="bv")
            nc.scalar.dma_start(bv_sb[:], bv[s])
            bk_sb = bpool.tile([1, DIM], F32R, tag="bk")
            nc.scalar.dma_start(bk_sb[:], bk[s])
            return x_t, a_t, bq_sb, bv_sb, bk_sb

        loaded = {0: load_sample(0)}

        # Resident transposed base weight: 24 tiles [128, 1024] (512 KB
        # DMAs), chunk-pair-major so early chunks land first.
        w_sb = {}
        for cp in range(NCH // 2):
            for k in range(KT):
                t = wpool.tile([128, 1024], F32R, tag=f"w{k}_{cp}")
                nc.sync.dma_start(
                    t[:],
                    wT[k * 128 : (k + 1) * 128, cp * 1024 : (cp + 1) * 1024],
                )
                w_sb[(k, cp)] = t

        for s in range(SPC):
            if s not in loaded:
                loaded[s] = load_sample(s)
            x_t, a_t, bq_sb, bv_sb, bk_sb = loaded.pop(s)

            def xk(k):
                return x_t[:, bass.ts(k, SEQ)]

            # t_q[32, 512] = A_q_cat.T @ x ; t_v likewise (separate PSUM
            # tiles so SBUF copies stay partition-aligned)
            pt_q = pst.tile([32, SEQ], F32, tag="ptq")
            pt_v = pst.tile([32, SEQ], F32, tag="ptv")
            for k in range(KT):
                nc.tensor.matmul(
                    pt_q[:],
                    a_t[:, k * 64 : k * 64 + 32],
                    xk(k),
                    start=(k == 0),
                    stop=(k == KT - 1),
                )
            for k in range(KT):
                nc.tensor.matmul(
                    pt_v[:],
                    a_t[:, k * 64 + 32 : (k + 1) * 64],
                    xk(k),
                    start=(k == 0),
                    stop=(k == KT - 1),
                )
            # tq/tv rows 0-31 = t, row 32 = ones (bias rides rank-1 row)
            tq_sb = tpool.tile([33, SEQ], F32R, tag="tq")
            tv_sb = tpool.tile([33, SEQ], F32R, tag="tv")
            ones_sb = tpool.tile([1, SEQ], F32R, tag="ones1")
            nc.scalar.dma_start(tq_sb[32:33, :], ones[0:1, :])
            nc.scalar.dma_start(tv_sb[32:33, :], ones[0:1, :])
            nc.scalar.dma_start(ones_sb[:], ones[1:2, :])
            nc.vector.tensor_copy(tq_sb[0:32, :], pt_q[:])
            nc.vector.tensor_copy(tv_sb[0:32, :], pt_v[:])

            # chunk-pair-outer so the base-weight stream is consumed at
            # DMA pace; stores batched per [128, 1024] chunk-pair
            for cp in range(NCH // 2):
                for nt in range(NT):
                    tok = bass.ts(nt, 128)
                    o_sb = opool.tile([128, 1024], F32, tag="o")
                    for half in range(2):
                        c = cp * 2 + half
                        po = pso.tile([128, 512], F32, tag="po")
                        # small-K LoRA/bias matmul opens the group so the
                        # group-boundary issue penalty lands on a cheap op
                        if c < 2:
                            nc.tensor.matmul(
                                po[:],
                                tq_sb[:, tok],
                                bq_sb[:, bass.ts(c, 512)],
                                start=True,
                                stop=False,
                            )
                        elif c >= 4:
                            nc.tensor.matmul(
                                po[:],
                                tv_sb[:, tok],
                                bv_sb[:, bass.ts(c - 4, 512)],
                                start=True,
                                stop=False,
                            )
                        else:
                            nc.tensor.matmul(
                                po[:],
                                ones_sb[:, tok],
                                bk_sb[:, bass.ts(c - 2, 512)],
                                start=True,
                                stop=False,
                            )
                        for k in range(KT):
                            nc.tensor.matmul(
                                po[:],
                                x_t[:, k * SEQ + nt * 128 : k * SEQ + (nt + 1) * 128],
                                w_sb[(k, cp)][:, bass.ts(half, 512)],
                                start=False,
                                stop=(k == KT - 1),
                            )
                        if (c * NT + nt) % 2 == 0:
                            nc.vector.tensor_copy(
                                o_sb[:, bass.ts(half, 512)], po[:]
                            )
                        else:
                            nc.scalar.copy(o_sb[:, bass.ts(half, 512)], po[:])
                    nc.scalar.dma_start(
                        out[s, tok, bass.ts(cp, 1024)], o_sb[:]
                    )

    nc.compile()
    return nc


def _get_nc():
    if "nc" not in _NC_CACHE:
        _NC_CACHE["nc"] = _build_nc()
    return _NC_CACHE["nc"]


def kernel(**inputs):
    x = np.asarray(inputs["x"], dtype=np.float32)
    weight = np.asarray(inputs["weight"], dtype=np.float32)
    bias = np.asarray(inputs["bias"], dtype=np.float32)
    A_q = np.asarray(inputs["A_q_pool"], dtype=np.float32)
    B_q = np.asarray(inputs["B_q_pool"], dtype=np.float32)
    A_v = np.asarray(inputs["A_v_pool"], dtype=np.float32)
    B_v = np.asarray(inputs["B_v_pool"], dtype=np.float32)
    bias_pool = np.asarray(inputs["bias_pool"], dtype=np.float32)
    attn = np.asarray(inputs["attn"], dtype=np.float32)
    idx = np.asarray(inputs["idx"]).astype(np.int64)

    # -- host-side prep (cheap; pools are tiny) --
    xT = np.ascontiguousarray(x.transpose(0, 2, 1))  # [B, DIM, SEQ]
    wT = np.ascontiguousarray(weight.T)  # [DIM, 3*DIM]

    i0, i1 = idx[:, 0], idx[:, 1]
    # [B, DIM, 64]: columns 0-15 q/k0, 16-31 q/k1, 32-47 v/k0, 48-63 v/k1
    acat = np.concatenate([A_q[i0], A_q[i1], A_v[i0], A_v[i1]], axis=2)
    acat = np.ascontiguousarray(acat)

    wgt = (SCALE * attn)[:, :, None, None]  # [B, K, 1, 1]
    bq_s = (B_q[idx] * wgt).reshape(BSZ, TOPK * RANK, DIM)
    bv_s = (B_v[idx] * wgt).reshape(BSZ, TOPK * RANK, DIM)
    bias_total = bias[None, :] + SCALE * np.einsum(
        "bko,bk->bo", bias_pool[idx], attn
    )  # [B, 3*DIM]
    bq_ext = np.ascontiguousarray(
        np.concatenate([bq_s, bias_total[:, None, :DIM]], axis=1)
    )  # [B, 33, DIM]
    bv_ext = np.ascontiguousarray(
        np.concatenate([bv_s, bias_total[:, None, 2 * DIM :]], axis=1)
    )  # [B, 33, DIM]
    bk_row = np.ascontiguousarray(bias_total[:, None, DIM : 2 * DIM])  # [B, 1, DIM]
    ones = np.ones((2, SEQ), dtype=np.float32)

    nc = _get_nc()
    in_maps = []
    for c in range(NCORES):
        sl = slice(c * SPC, (c + 1) * SPC)
        in_maps.append(
            {
                "xT": xT[sl],
                "wT": wT,
                "acat": acat[sl],
                "bq": bq_ext[sl],
                "bv": bv_ext[sl],
                "bk": bk_row[sl],
                "ones": ones,
            }
        )
    global _LAST_IN_MAPS
    _LAST_IN_MAPS = in_maps
    res = run_bass_kernel_spmd(nc, in_maps, list(range(NCORES)))
    out = np.concatenate(
        [np.asarray(res.results[i]["out"]) for i in range(NCORES)], axis=0
    )
    return out.astype(np.float32, copy=False)


_LAST_IN_MAPS = None


# revision 18
# speedup vs baseline: 1.1458x; 1.0218x over previous
"""Trainium2 Bass kernel for MoE-LoRA fused QKV projection.

Computes, for x[32,512,1024], weight[3072,1024], per-sample top-2 LoRA
expert pools (rank 16) and expert biases:

    qkv = x @ W.T + bias
    qkv[..., :1024]  += SCALE * sum_k attn[b,k] * (x @ A_q[idx]) @ B_q[idx]
    qkv[..., 2048:]  += SCALE * sum_k attn[b,k] * (x @ A_v[idx]) @ B_v[idx]
    qkv += SCALE * sum_k attn[b,k] * bias_pool[idx]

Strategy: data-parallel over batch on 8 NeuronCores (4 samples/core).
Host side: transpose x / weight, gather + scale the tiny LoRA pools per
sample, fold all bias terms into one per-sample row vector that rides the
LoRA matmul via an appended ones-row (rank-1 matmul trick). Device side:
everything is dense fp32r (TF32-like, full PE rate) matmuls accumulated
in PSUM.
"""

import sys

for _p in ("/opt/trn_rl_repo",):
    if _p not in sys.path:
        sys.path.append(_p)

from contextlib import ExitStack

import numpy as np

import concourse.bass as bass
import concourse.tile as tile
from concourse import bacc, mybir
from concourse.bass_utils import run_bass_kernel_spmd

DIM = 1024
RANK = 16
TOPK = 2
BSZ = 32
SEQ = 512
SCALE = 1.0
NCORES = 8
SPC = BSZ // NCORES  # samples per core
NT = SEQ // 128  # token tiles per sample
KT = DIM // 128  # contraction tiles
NC3 = 3 * DIM
NCH = NC3 // 512  # output column chunks

F32 = mybir.dt.float32
F32R = mybir.dt.float32r

_NC_CACHE = {}


def _build_nc():
    nc = bacc.Bacc("TRN2", target_bir_lowering=False, debug=False)
    xT = nc.dram_tensor("xT", [SPC, DIM, SEQ], F32R, kind="ExternalInput").ap()
    wT = nc.dram_tensor("wT", [DIM, NC3], F32R, kind="ExternalInput").ap()
    acat = nc.dram_tensor("acat", [SPC, DIM, 4 * RANK], F32R, kind="ExternalInput").ap()
    bq = nc.dram_tensor("bq", [SPC, 33, DIM], F32R, kind="ExternalInput").ap()
    bv = nc.dram_tensor("bv", [SPC, 33, DIM], F32R, kind="ExternalInput").ap()
    bk = nc.dram_tensor("bk", [SPC, 1, DIM], F32R, kind="ExternalInput").ap()
    ones = nc.dram_tensor("ones", [2, SEQ], F32R, kind="ExternalInput").ap()
    out = nc.dram_tensor("out", [SPC, SEQ, NC3], F32, kind="ExternalOutput").ap()

    with tile.TileContext(nc) as tc, ExitStack() as ctx:
        wpool = ctx.enter_context(tc.tile_pool(name="w", bufs=1))
        xpool = ctx.enter_context(tc.tile_pool(name="x", bufs=2))
        apool = ctx.enter_context(tc.tile_pool(name="a", bufs=2))
        bpool = ctx.enter_context(tc.tile_pool(name="b", bufs=2))
        tpool = ctx.enter_context(tc.tile_pool(name="t", bufs=2))
        opool = ctx.enter_context(tc.tile_pool(name="o", bufs=3))
        pst = ctx.enter_context(tc.tile_pool(name="pst", bufs=1, space="PSUM"))
        pso = ctx.enter_context(tc.tile_pool(name="pso", bufs=5, space="PSUM"))

        def load_sample(s):
            # A_cat first (tiny, unblocks the first t-matmul earliest)
            a_t = apool.tile([128, KT * 4 * RANK], F32R, tag="a")
            nc.sync.dma_start(
                a_t[:].rearrange("p (k r) -> p k r", r=4 * RANK),
                acat[s].rearrange("(k p) r -> p k r", p=128),
            )
            # x: four 512 KB DMAs [DIM, SEQ] -> [128, (k j)] so the first
            # t-matmuls start as soon as the first k-pair lands
            x_t = xpool.tile([128, KT * SEQ], F32R, tag="x")
            h = KT // 4
            for i in range(4):
                nc.sync.dma_start(
                    x_t[:, i * h * SEQ : (i + 1) * h * SEQ].rearrange(
                        "p (k j) -> p k j", j=SEQ
                    ),
                    xT[s][i * h * 128 : (i + 1) * h * 128, :].rearrange(
                        "(k p) j -> p k j", p=128
                    ),
                )
            bq_sb = bpool.tile([33, DIM], F32R, tag="bq")
            nc.scalar.dma_start(bq_sb[:], bq[s])
            bv_sb = bpool.tile([33, DIM], F32R, tag="bv")
            nc.scalar.dma_start(bv_sb[:], bv[s])
            bk_sb = bpool.tile([1, DIM], F32R, tag="bk")
            nc.scalar.dma_start(bk_sb[:], bk[s])
            return x_t, a_t, bq_sb, bv_sb, bk_sb

        loaded = {0: load_sample(0)}

        # Resident transposed base weight: 24 tiles [128, 1024] (512 KB
        # DMAs), chunk-pair-major so early chunks land first.
        w_sb = {}
        for cp in range(NCH // 2):
            for k in range(KT):
                t = wpool.tile([128, 1024], F32R, tag=f"w{k}_{cp}")
                nc.sync.dma_start(
                    t[:],
                    wT[k * 128 : (k + 1) * 128, cp * 1024 : (cp + 1) * 1024],
                )
                w_sb[(k, cp)] = t

        for s in range(SPC):
            if s not in loaded:
                loaded[s] = load_sample(s)
            x_t, a_t, bq_sb, bv_sb, bk_sb = loaded.pop(s)

            def xk(k):
                return x_t[:, bass.ts(k, SEQ)]

            # t_q[32, 512] = A_q_cat.T @ x ; t_v likewise (separate PSUM
            # tiles so SBUF copies stay partition-aligned)
            pt_q = pst.tile([32, SEQ], F32, tag="ptq")
            pt_v = pst.tile([32, SEQ], F32, tag="ptv")
            for k in range(KT):
                nc.tensor.matmul(
                    pt_q[:],
                    a_t[:, k * 64 : k * 64 + 32],
                    xk(k),
                    start=(k == 0),
                    stop=(k == KT - 1),
                )
            for k in range(KT):
                nc.tensor.matmul(
                    pt_v[:],
                    a_t[:, k * 64 + 32 : (k + 1) * 64],
                    xk(k),
                    start=(k == 0),
                    stop=(k == KT - 1),
                )
            # tq/tv rows 0-31 = t, row 32 = ones (bias rides rank-1 row)
            tq_sb = tpool.tile([33, SEQ], F32R, tag="tq")
            tv_sb = tpool.tile([33, SEQ], F32R, tag="tv")
            ones_sb = tpool.tile([1, SEQ], F32R, tag="ones1")
            nc.scalar.dma_start(tq_sb[32:33, :], ones[0:1, :])
            nc.scalar.dma_start(tv_sb[32:33, :], ones[0:1, :])
            nc.scalar.dma_start(ones_sb[:], ones[1:2, :])
            nc.vector.tensor_copy(tq_sb[0:32, :], pt_q[:])
            nc.vector.tensor_copy(tv_sb[0:32, :], pt_v[:])

            # chunk-pair-outer so the base-weight stream is consumed at
            # DMA pace; stores batched per [128, 1024] chunk-pair
            for cp in range(NCH // 2):
                for nt in range(NT):
                    tok = bass.ts(nt, 128)
                    o_sb = opool.tile([128, 1024], F32, tag="o")
                    for half in range(2):
                        c = cp * 2 + half
                        po = pso.tile([128, 512], F32, tag="po")
                        # small-K LoRA/bias matmul opens the group so the
                        # group-boundary issue penalty lands on a cheap op
                        if c < 2:
                            nc.tensor.matmul(
                                po[:],
                                tq_sb[:, tok],
                                bq_sb[:, bass.ts(c, 512)],
                                start=True,
                                stop=False,
                            )
                        elif c >= 4:
                            nc.tensor.matmul(
                                po[:],
                                tv_sb[:, tok],
                                bv_sb[:, bass.ts(c - 4, 512)],
                                start=True,
                                stop=False,
                            )
                        else:
                            nc.tensor.matmul(
                                po[:],
                                ones_sb[:, tok],
                                bk_sb[:, bass.ts(c - 2, 512)],
                                start=True,
                                stop=False,
                            )
                        for k in range(KT):
                            nc.tensor.matmul(
                                po[:],
                                x_t[:, k * SEQ + nt * 128 : k * SEQ + (nt + 1) * 128],
                                w_sb[(k, cp)][:, bass.ts(half, 512)],
                                start=False,
                                stop=(k == KT - 1),
                            )
                        # split the PSUM evacuation across DVE and ACT so
                        # the PSUM-read window (which slows concurrent PE
                        # writes) is as short as possible
                        nc.vector.tensor_copy(
                            o_sb[:, half * 512 : half * 512 + 256],
                            po[:, 0:256],
                        )
                        nc.scalar.copy(
                            o_sb[:, half * 512 + 256 : half * 512 + 512],
                            po[:, 256:512],
                        )
                    nc.scalar.dma_start(
                        out[s, tok, bass.ts(cp, 1024)], o_sb[:]
                    )

    nc.compile()
    return nc


def _get_nc():
    if "nc" not in _NC_CACHE:
        _NC_CACHE["nc"] = _build_nc()
    return _NC_CACHE["nc"]


def kernel(**inputs):
    x = np.asarray(inputs["x"], dtype=np.float32)
    weight = np.asarray(inputs["weight"], dtype=np.float32)
    bias = np.asarray(inputs["bias"], dtype=np.float32)
    A_q = np.asarray(inputs["A_q_pool"], dtype=np.float32)
    B_q = np.asarray(inputs["B_q_pool"], dtype=np.float32)
    A_v = np.asarray(inputs["A_v_pool"], dtype=np.float32)
    B_v = np.asarray(inputs["B_v_pool"], dtype=np.float32)
    bias_pool = np.asarray(inputs["bias_pool"], dtype=np.float32)
    attn = np.asarray(inputs["attn"], dtype=np.float32)
    idx = np.asarray(inputs["idx"]).astype(np.int64)

    # -- host-side prep (cheap; pools are tiny) --
    xT = np.ascontiguousarray(x.transpose(0, 2, 1))  # [B, DIM, SEQ]
    wT = np.ascontiguousarray(weight.T)  # [DIM, 3*DIM]

    i0, i1 = idx[:, 0], idx[:, 1]
    # [B, DIM, 64]: columns 0-15 q/k0, 16-31 q/k1, 32-47 v/k0, 48-63 v/k1
    acat = np.concatenate([A_q[i0], A_q[i1], A_v[i0], A_v[i1]], axis=2)
    acat = np.ascontiguousarray(acat)

    wgt = (SCALE * attn)[:, :, None, None]  # [B, K, 1, 1]
    bq_s = (B_q[idx] * wgt).reshape(BSZ, TOPK * RANK, DIM)
    bv_s = (B_v[idx] * wgt).reshape(BSZ, TOPK * RANK, DIM)
    bias_total = bias[None, :] + SCALE * np.einsum(
        "bko,bk->bo", bias_pool[idx], attn
    )  # [B, 3*DIM]
    bq_ext = np.ascontiguousarray(
        np.concatenate([bq_s, bias_total[:, None, :DIM]], axis=1)
    )  # [B, 33, DIM]
    bv_ext = np.ascontiguousarray(
        np.concatenate([bv_s, bias_total[:, None, 2 * DIM :]], axis=1)
    )  # [B, 33, DIM]
    bk_row = np.ascontiguousarray(bias_total[:, None, DIM : 2 * DIM])  # [B, 1, DIM]
    ones = np.ones((2, SEQ), dtype=np.float32)

    nc = _get_nc()
    in_maps = []
    for c in range(NCORES):
        sl = slice(c * SPC, (c + 1) * SPC)
        in_maps.append(
            {
                "xT": xT[sl],
                "wT": wT,
                "acat": acat[sl],
                "bq": bq_ext[sl],
                "bv": bv_ext[sl],
                "bk": bk_row[sl],
                "ones": ones,
            }
        )
    global _LAST_IN_MAPS
    _LAST_IN_MAPS = in_maps
    res = run_bass_kernel_spmd(nc, in_maps, list(range(NCORES)))
    out = np.concatenate(
        [np.asarray(res.results[i]["out"]) for i in range(NCORES)], axis=0
    )
    return out.astype(np.float32, copy=False)


_LAST_IN_MAPS = None


# revision 27
# speedup vs baseline: 1.1898x; 1.0384x over previous
"""Trainium2 Bass kernel for MoE-LoRA fused QKV projection.

Computes, for x[32,512,1024], weight[3072,1024], per-sample top-2 LoRA
expert pools (rank 16) and expert biases:

    qkv = x @ W.T + bias
    qkv[..., :1024]  += SCALE * sum_k attn[b,k] * (x @ A_q[idx]) @ B_q[idx]
    qkv[..., 2048:]  += SCALE * sum_k attn[b,k] * (x @ A_v[idx]) @ B_v[idx]
    qkv += SCALE * sum_k attn[b,k] * bias_pool[idx]

Strategy: data-parallel over batch on 8 NeuronCores (4 samples/core).
Host side: transpose x / weight, gather + scale the tiny LoRA pools per
sample, fold all bias terms into one per-sample row vector that rides the
LoRA matmul via an appended ones-row (rank-1 matmul trick). Device side:
everything is dense fp32r (TF32-like, full PE rate) matmuls accumulated
in PSUM.
"""

import sys

for _p in ("/opt/trn_rl_repo",):
    if _p not in sys.path:
        sys.path.append(_p)

from contextlib import ExitStack

import numpy as np

import concourse.bass as bass
import concourse.tile as tile
from concourse import bacc, mybir
from concourse.bass_utils import run_bass_kernel_spmd

DIM = 1024
RANK = 16
TOPK = 2
BSZ = 32
SEQ = 512
SCALE = 1.0
NCORES = 8
SPC = BSZ // NCORES  # samples per core
NT = SEQ // 128  # token tiles per sample
KT = DIM // 128  # contraction tiles
NC3 = 3 * DIM
NCH = NC3 // 512  # output column chunks

F32 = mybir.dt.float32
F32R = mybir.dt.float32r

_NC_CACHE = {}


def _build_nc():
    nc = bacc.Bacc("TRN2", target_bir_lowering=False, debug=False)
    xT = nc.dram_tensor("xT", [SPC, DIM, SEQ], F32R, kind="ExternalInput").ap()
    wT = nc.dram_tensor("wT", [DIM, NC3], F32R, kind="ExternalInput").ap()
    acat = nc.dram_tensor("acat", [SPC, DIM, 4 * RANK], F32R, kind="ExternalInput").ap()
    bq = nc.dram_tensor("bq", [SPC, 33, DIM], F32R, kind="ExternalInput").ap()
    bv = nc.dram_tensor("bv", [SPC, 33, DIM], F32R, kind="ExternalInput").ap()
    bk = nc.dram_tensor("bk", [SPC, 128, DIM], F32, kind="ExternalInput").ap()
    ones = nc.dram_tensor("ones", [2, SEQ], F32R, kind="ExternalInput").ap()
    out = nc.dram_tensor("out", [SPC, SEQ, NC3], F32, kind="ExternalOutput").ap()

    with tile.TileContext(nc) as tc, ExitStack() as ctx:
        wpool = ctx.enter_context(tc.tile_pool(name="w", bufs=1))
        xpool = ctx.enter_context(tc.tile_pool(name="x", bufs=2))
        apool = ctx.enter_context(tc.tile_pool(name="a", bufs=2))
        bpool = ctx.enter_context(tc.tile_pool(name="b", bufs=2))
        tpool = ctx.enter_context(tc.tile_pool(name="t", bufs=2))
        opool = ctx.enter_context(tc.tile_pool(name="o", bufs=3))
        pst = ctx.enter_context(tc.tile_pool(name="pst", bufs=1, space="PSUM"))
        pso = ctx.enter_context(tc.tile_pool(name="pso", bufs=5, space="PSUM"))

        def load_sample(s):
            # A_cat first (tiny, unblocks the first t-matmul earliest)
            a_t = apool.tile([128, KT * 4 * RANK], F32R, tag="a")
            nc.sync.dma_start(
                a_t[:].rearrange("p (k r) -> p k r", r=4 * RANK),
                acat[s].rearrange("(k p) r -> p k r", p=128),
            )
            # x: four 512 KB DMAs [DIM, SEQ] -> [128, (k j)] so the first
            # t-matmuls start as soon as the first k-pair lands
            x_t = xpool.tile([128, KT * SEQ], F32R, tag="x")
            h = KT // 4
            for i in range(4):
                nc.sync.dma_start(
                    x_t[:, i * h * SEQ : (i + 1) * h * SEQ].rearrange(
                        "p (k j) -> p k j", j=SEQ
                    ),
                    xT[s][i * h * 128 : (i + 1) * h * 128, :].rearrange(
                        "(k p) j -> p k j", p=128
                    ),
                )
            bq_sb = bpool.tile([33, DIM], F32R, tag="bq")
            nc.scalar.dma_start(bq_sb[:], bq[s])
            bv_sb = bpool.tile([33, DIM], F32R, tag="bv")
            nc.scalar.dma_start(bv_sb[:], bv[s])
            # k-part bias pre-broadcast across all 128 partitions: added
            # during PSUM evacuation (saves a PE matmul per k-chunk group)
            bk_sb = bpool.tile([128, DIM], F32, tag="bk")
            nc.scalar.dma_start(bk_sb[:], bk[s])
            return x_t, a_t, bq_sb, bv_sb, bk_sb

        loaded = {0: load_sample(0)}

        # Resident transposed base weight: 24 tiles [128, 1024] (512 KB
        # DMAs), chunk-pair-major so early chunks land first.
        w_sb = {}
        for cp in range(NCH // 2):
            for k in range(KT):
                t = wpool.tile([128, 1024], F32R, tag=f"w{k}_{cp}")
                nc.sync.dma_start(
                    t[:],
                    wT[k * 128 : (k + 1) * 128, cp * 1024 : (cp + 1) * 1024],
                )
                w_sb[(k, cp)] = t

        for s in range(SPC):
            if s not in loaded:
                loaded[s] = load_sample(s)
            x_t, a_t, bq_sb, bv_sb, bk_sb = loaded.pop(s)

            def xk(k):
                return x_t[:, bass.ts(k, SEQ)]

            # t_q[32, 512] = A_q_cat.T @ x ; t_v likewise (separate PSUM
            # tiles: a 33-partition matmul operand must start at base 0,
            # and copies must stay partition-aligned)
            pt_q = pst.tile([32, SEQ], F32, tag="ptq")
            pt_v = pst.tile([32, SEQ], F32, tag="ptv")
            for k in range(KT):
                nc.tensor.matmul(
                    pt_q[:],
                    a_t[:, k * 64 : k * 64 + 32],
                    xk(k),
                    start=(k == 0),
                    stop=(k == KT - 1),
                )
            for k in range(KT):
                nc.tensor.matmul(
                    pt_v[:],
                    a_t[:, k * 64 + 32 : (k + 1) * 64],
                    xk(k),
                    start=(k == 0),
                    stop=(k == KT - 1),
                )
            # tq/tv rows 0-31 = t, row 32 = ones (bias rides rank-1 row)
            tq_sb = tpool.tile([33, SEQ], F32R, tag="tq")
            tv_sb = tpool.tile([33, SEQ], F32R, tag="tv")
            nc.scalar.dma_start(tq_sb[32:33, :], ones[0:1, :])
            nc.scalar.dma_start(tv_sb[32:33, :], ones[0:1, :])
            nc.vector.tensor_copy(tq_sb[0:32, :], pt_q[:])
            nc.vector.tensor_copy(tv_sb[0:32, :], pt_v[:])

            # chunk-pair-outer so the base-weight stream is consumed at
            # DMA pace; stores batched per [128, 1024] chunk-pair
            for cp in range(NCH // 2):
                for nt in range(NT):
                    tok = bass.ts(nt, 128)
                    o_sb = opool.tile([128, 1024], F32, tag="o")
                    for half in range(2):
                        c = cp * 2 + half
                        po = pso.tile([128, 512], F32, tag="po")
                        # small-K LoRA matmul opens the group so the
                        # group-boundary issue penalty lands on a cheap op
                        first = True
                        if c < 2:
                            nc.tensor.matmul(
                                po[:],
                                tq_sb[:, tok],
                                bq_sb[:, bass.ts(c, 512)],
                                start=True,
                                stop=False,
                            )
                            first = False
                        elif c >= 4:
                            nc.tensor.matmul(
                                po[:],
                                tv_sb[:, tok],
                                bv_sb[:, bass.ts(c - 4, 512)],
                                start=True,
                                stop=False,
                            )
                            first = False
                        for k in range(KT):
                            nc.tensor.matmul(
                                po[:],
                                x_t[:, k * SEQ + nt * 128 : k * SEQ + (nt + 1) * 128],
                                w_sb[(k, cp)][:, bass.ts(half, 512)],
                                start=first and k == 0,
                                stop=(k == KT - 1),
                            )
                        if 2 <= c < 4:
                            # k-part: bias added during evacuation (DVE)
                            nc.vector.tensor_add(
                                o_sb[:, bass.ts(half, 512)],
                                po[:],
                                bk_sb[:, bass.ts(c - 2, 512)],
                            )
                        else:
                            # split the PSUM evacuation across DVE and ACT
                            # to shorten the PSUM-read window
                            nc.vector.tensor_copy(
                                o_sb[:, half * 512 : half * 512 + 256],
                                po[:, 0:256],
                            )
                            nc.scalar.copy(
                                o_sb[:, half * 512 + 256 : half * 512 + 512],
                                po[:, 256:512],
                            )
                    nc.scalar.dma_start(
                        out[s, tok, bass.ts(cp, 1024)], o_sb[:]
                    )

    nc.compile()
    return nc


def _get_nc():
    if "nc" not in _NC_CACHE:
        _NC_CACHE["nc"] = _build_nc()
    return _NC_CACHE["nc"]


def kernel(**inputs):
    x = np.asarray(inputs["x"], dtype=np.float32)
    weight = np.asarray(inputs["weight"], dtype=np.float32)
    bias = np.asarray(inputs["bias"], dtype=np.float32)
    A_q = np.asarray(inputs["A_q_pool"], dtype=np.float32)
    B_q = np.asarray(inputs["B_q_pool"], dtype=np.float32)
    A_v = np.asarray(inputs["A_v_pool"], dtype=np.float32)
    B_v = np.asarray(inputs["B_v_pool"], dtype=np.float32)
    bias_pool = np.asarray(inputs["bias_pool"], dtype=np.float32)
    attn = np.asarray(inputs["attn"], dtype=np.float32)
    idx = np.asarray(inputs["idx"]).astype(np.int64)

    # -- host-side prep (cheap; pools are tiny) --
    xT = np.ascontiguousarray(x.transpose(0, 2, 1))  # [B, DIM, SEQ]
    wT = np.ascontiguousarray(weight.T)  # [DIM, 3*DIM]

    i0, i1 = idx[:, 0], idx[:, 1]
    # [B, DIM, 64]: columns 0-15 q/k0, 16-31 q/k1, 32-47 v/k0, 48-63 v/k1
    acat = np.concatenate([A_q[i0], A_q[i1], A_v[i0], A_v[i1]], axis=2)
    acat = np.ascontiguousarray(acat)

    wgt = (SCALE * attn)[:, :, None, None]  # [B, K, 1, 1]
    bq_s = (B_q[idx] * wgt).reshape(BSZ, TOPK * RANK, DIM)
    bv_s = (B_v[idx] * wgt).reshape(BSZ, TOPK * RANK, DIM)
    bias_total = bias[None, :] + SCALE * np.einsum(
        "bko,bk->bo", bias_pool[idx], attn
    )  # [B, 3*DIM]
    bq_ext = np.ascontiguousarray(
        np.concatenate([bq_s, bias_total[:, None, :DIM]], axis=1)
    )  # [B, 33, DIM]
    bv_ext = np.ascontiguousarray(
        np.concatenate([bv_s, bias_total[:, None, 2 * DIM :]], axis=1)
    )  # [B, 33, DIM]
    # k-part bias replicated across the 128 token partitions
    bk_row = np.ascontiguousarray(
        np.broadcast_to(bias_total[:, None, DIM : 2 * DIM], (BSZ, 128, DIM))
    )  # [B, 128, DIM]
    ones = np.ones((2, SEQ), dtype=np.float32)

    nc = _get_nc()
    in_maps = []
    for c in range(NCORES):
        sl = slice(c * SPC, (c + 1) * SPC)
        in_maps.append(
            {
                "xT": xT[sl],
                "wT": wT,
                "acat": acat[sl],
                "bq": bq_ext[sl],
                "bv": bv_ext[sl],
                "bk": bk_row[sl],
                "ones": ones,
            }
        )
    global _LAST_IN_MAPS
    _LAST_IN_MAPS = in_maps
    res = run_bass_kernel_spmd(nc, in_maps, list(range(NCORES)))
    out = np.concatenate(
        [np.asarray(res.results[i]["out"]) for i in range(NCORES)], axis=0
    )
    return out.astype(np.float32, copy=False)


_LAST_IN_MAPS = None


# revision 33
# speedup vs baseline: 1.1975x; 1.0065x over previous
"""Trainium2 Bass kernel for MoE-LoRA fused QKV projection.

Computes, for x[32,512,1024], weight[3072,1024], per-sample top-2 LoRA
expert pools (rank 16) and expert biases:

    qkv = x @ W.T + bias
    qkv[..., :1024]  += SCALE * sum_k attn[b,k] * (x @ A_q[idx]) @ B_q[idx]
    qkv[..., 2048:]  += SCALE * sum_k attn[b,k] * (x @ A_v[idx]) @ B_v[idx]
    qkv += SCALE * sum_k attn[b,k] * bias_pool[idx]

Strategy: data-parallel over batch on 8 NeuronCores (4 samples/core).
Host side: transpose x / weight, gather + scale the tiny LoRA pools per
sample, fold all bias terms into one per-sample row vector that rides the
LoRA matmul via an appended ones-row (rank-1 matmul trick). Device side:
everything is dense fp32r (TF32-like, full PE rate) matmuls accumulated
in PSUM.
"""

import sys

for _p in ("/opt/trn_rl_repo",):
    if _p not in sys.path:
        sys.path.append(_p)

from contextlib import ExitStack

import numpy as np

import concourse.bass as bass
import concourse.tile as tile
from concourse import bacc, mybir
from concourse.bass_utils import run_bass_kernel_spmd

DIM = 1024
RANK = 16
TOPK = 2
BSZ = 32
SEQ = 512
SCALE = 1.0
NCORES = 8
SPC = BSZ // NCORES  # samples per core
NT = SEQ // 128  # token tiles per sample
KT = DIM // 128  # contraction tiles
NC3 = 3 * DIM
NCH = NC3 // 512  # output column chunks

F32 = mybir.dt.float32
F32R = mybir.dt.float32r

_NC_CACHE = {}


def _build_nc():
    nc = bacc.Bacc("TRN2", target_bir_lowering=False, debug=False)
    # all large inputs are pre-packed host-side into exact SBUF layouts so
    # every DMA is 2D-contiguous with multi-KB descriptors
    xT = nc.dram_tensor("xT", [SPC, 128, KT * SEQ], F32R, kind="ExternalInput").ap()
    wT = nc.dram_tensor("wT", [128, 3 * KT * 1024], F32R, kind="ExternalInput").ap()
    acat = nc.dram_tensor(
        "acat", [SPC, 128, KT * 4 * RANK], F32R, kind="ExternalInput"
    ).ap()
    bq = nc.dram_tensor("bq", [SPC, 33, DIM], F32R, kind="ExternalInput").ap()
    bv = nc.dram_tensor("bv", [SPC, 33, DIM], F32R, kind="ExternalInput").ap()
    bk = nc.dram_tensor("bk", [SPC, 128, DIM], F32, kind="ExternalInput").ap()
    ones = nc.dram_tensor("ones", [2, SEQ], F32R, kind="ExternalInput").ap()
    out = nc.dram_tensor("out", [SPC, SEQ, NC3], F32, kind="ExternalOutput").ap()

    with tile.TileContext(nc) as tc, ExitStack() as ctx:
        wpool = ctx.enter_context(tc.tile_pool(name="w", bufs=1))
        xpool = ctx.enter_context(tc.tile_pool(name="x", bufs=2))
        apool = ctx.enter_context(tc.tile_pool(name="a", bufs=2))
        bpool = ctx.enter_context(tc.tile_pool(name="b", bufs=2))
        tpool = ctx.enter_context(tc.tile_pool(name="t", bufs=2))
        opool = ctx.enter_context(tc.tile_pool(name="o", bufs=3))
        pst = ctx.enter_context(tc.tile_pool(name="pst", bufs=1, space="PSUM"))
        pso = ctx.enter_context(tc.tile_pool(name="pso", bufs=5, space="PSUM"))

        def load_sample(s):
            # A_cat first (tiny, unblocks the first t-matmul earliest)
            a_t = apool.tile([128, KT * 4 * RANK], F32R, tag="a")
            nc.sync.dma_start(a_t[:], acat[s])
            # x in two 1 MB halves so the first t-matmuls start early
            x_t = xpool.tile([128, KT * SEQ], F32R, tag="x")
            hx = KT * SEQ // 2
            for i in range(2):
                nc.sync.dma_start(
                    x_t[:, i * hx : (i + 1) * hx], xT[s, :, i * hx : (i + 1) * hx]
                )
            bq_sb = bpool.tile([33, DIM], F32R, tag="bq")
            nc.scalar.dma_start(bq_sb[:], bq[s])
            bv_sb = bpool.tile([33, DIM], F32R, tag="bv")
            nc.scalar.dma_start(bv_sb[:], bv[s])
            # k-part bias pre-broadcast across all 128 partitions: added
            # during PSUM evacuation (saves a PE matmul per k-chunk group)
            bk_sb = bpool.tile([128, DIM], F32, tag="bk")
            nc.scalar.dma_start(bk_sb[:], bk[s])
            return x_t, a_t, bq_sb, bv_sb, bk_sb

        loaded = {0: load_sample(0)}

        # Resident base weight: three 4 MB chunk-pair DMAs into one tile;
        # free layout (cp, k, col): w for (k, cp) at cp*8*1024 + k*1024
        w_all = wpool.tile([128, 3 * KT * 1024], F32R, tag="wall")
        wq = KT * 1024
        for cp in range(NCH // 2):
            nc.sync.dma_start(
                w_all[:, cp * wq : (cp + 1) * wq], wT[:, cp * wq : (cp + 1) * wq]
            )

        def wtile(k, cp):
            off = cp * wq + k * 1024
            return w_all[:, off : off + 1024]

        for s in range(SPC):
            if s not in loaded:
                loaded[s] = load_sample(s)
            x_t, a_t, bq_sb, bv_sb, bk_sb = loaded.pop(s)

            def xk(k):
                return x_t[:, bass.ts(k, SEQ)]

            # t_q[32, 512] = A_q_cat.T @ x ; t_v likewise (separate PSUM
            # tiles: a 33-partition matmul operand must start at base 0,
            # and copies must stay partition-aligned)
            pt_q = pst.tile([32, SEQ], F32, tag="ptq")
            pt_v = pst.tile([32, SEQ], F32, tag="ptv")
            for k in range(KT):
                nc.tensor.matmul(
                    pt_q[:],
                    a_t[:, k * 64 : k * 64 + 32],
                    xk(k),
                    start=(k == 0),
                    stop=(k == KT - 1),
                )
            for k in range(KT):
                nc.tensor.matmul(
                    pt_v[:],
                    a_t[:, k * 64 + 32 : (k + 1) * 64],
                    xk(k),
                    start=(k == 0),
                    stop=(k == KT - 1),
                )
            # tq/tv rows 0-31 = t, row 32 = ones (bias rides rank-1 row)
            tq_sb = tpool.tile([33, SEQ], F32R, tag="tq")
            tv_sb = tpool.tile([33, SEQ], F32R, tag="tv")
            nc.scalar.dma_start(tq_sb[32:33, :], ones[0:1, :])
            nc.scalar.dma_start(tv_sb[32:33, :], ones[0:1, :])
            nc.vector.tensor_copy(tq_sb[0:32, :], pt_q[:])
            nc.vector.tensor_copy(tv_sb[0:32, :], pt_v[:])

            # chunk-pair-outer so the base-weight stream is consumed at
            # DMA pace; stores batched per [128, 1024] chunk-pair
            for cp in range(NCH // 2):
                for nt in range(NT):
                    tok = bass.ts(nt, 128)
                    o_sb = opool.tile([128, 1024], F32, tag="o")
                    for half in range(2):
                        c = cp * 2 + half
                        po = pso.tile([128, 512], F32, tag="po")
                        # small-K LoRA matmul opens the group so the
                        # group-boundary issue penalty lands on a cheap op
                        first = True
                        if c < 2:
                            nc.tensor.matmul(
                                po[:],
                                tq_sb[:, tok],
                                bq_sb[:, bass.ts(c, 512)],
                                start=True,
                                stop=False,
                            )
                            first = False
                        elif c >= 4:
                            nc.tensor.matmul(
                                po[:],
                                tv_sb[:, tok],
                                bv_sb[:, bass.ts(c - 4, 512)],
                                start=True,
                                stop=False,
                            )
                            first = False
                        for k in range(KT):
                            wt = wtile(k, cp)
                            nc.tensor.matmul(
                                po[:],
                                x_t[:, k * SEQ + nt * 128 : k * SEQ + (nt + 1) * 128],
                                wt[:, bass.ts(half, 512)],
                                start=first and k == 0,
                                stop=(k == KT - 1),
                            )
                        if 2 <= c < 4:
                            # k-part: bias added during evacuation (DVE)
                            nc.vector.tensor_add(
                                o_sb[:, bass.ts(half, 512)],
                                po[:],
                                bk_sb[:, bass.ts(c - 2, 512)],
                            )
                        else:
                            # split the PSUM evacuation across DVE and ACT
                            # to shorten the PSUM-read window
                            nc.vector.tensor_copy(
                                o_sb[:, half * 512 : half * 512 + 256],
                                po[:, 0:256],
                            )
                            nc.scalar.copy(
                                o_sb[:, half * 512 + 256 : half * 512 + 512],
                                po[:, 256:512],
                            )
                    nc.scalar.dma_start(
                        out[s, tok, bass.ts(cp, 1024)], o_sb[:]
                    )

    nc.compile()
    return nc


def _get_nc():
    if "nc" not in _NC_CACHE:
        _NC_CACHE["nc"] = _build_nc()
    return _NC_CACHE["nc"]


def kernel(**inputs):
    x = np.asarray(inputs["x"], dtype=np.float32)
    weight = np.asarray(inputs["weight"], dtype=np.float32)
    bias = np.asarray(inputs["bias"], dtype=np.float32)
    A_q = np.asarray(inputs["A_q_pool"], dtype=np.float32)
    B_q = np.asarray(inputs["B_q_pool"], dtype=np.float32)
    A_v = np.asarray(inputs["A_v_pool"], dtype=np.float32)
    B_v = np.asarray(inputs["B_v_pool"], dtype=np.float32)
    bias_pool = np.asarray(inputs["bias_pool"], dtype=np.float32)
    attn = np.asarray(inputs["attn"], dtype=np.float32)
    idx = np.asarray(inputs["idx"]).astype(np.int64)

    # -- host-side prep: pack the big operands into exact SBUF layouts --
    # x[b, n, k*128+p] -> xT[b, p, k*SEQ + n]
    xT = np.ascontiguousarray(
        x.reshape(BSZ, SEQ, KT, 128).transpose(0, 3, 2, 1).reshape(BSZ, 128, KT * SEQ)
    )
    # weight.T[k*128+p, cp*1024+col] -> wT[p, cp*8192 + k*1024 + col]
    wT = np.ascontiguousarray(
        weight.T.reshape(KT, 128, 3, 1024)
        .transpose(1, 2, 0, 3)
        .reshape(128, 3 * KT * 1024)
    )

    i0, i1 = idx[:, 0], idx[:, 1]
    # [B, DIM, 64]: columns 0-15 q/k0, 16-31 q/k1, 32-47 v/k0, 48-63 v/k1
    acat = np.concatenate([A_q[i0], A_q[i1], A_v[i0], A_v[i1]], axis=2)
    # acat[b, k*128+p, r] -> [b, p, k*64 + r]
    acat = np.ascontiguousarray(
        acat.reshape(BSZ, KT, 128, 4 * RANK)
        .transpose(0, 2, 1, 3)
        .reshape(BSZ, 128, KT * 4 * RANK)
    )

    wgt = (SCALE * attn)[:, :, None, None]  # [B, K, 1, 1]
    bq_s = (B_q[idx] * wgt).reshape(BSZ, TOPK * RANK, DIM)
    bv_s = (B_v[idx] * wgt).reshape(BSZ, TOPK * RANK, DIM)
    bias_total = bias[None, :] + SCALE * np.einsum(
        "bko,bk->bo", bias_pool[idx], attn
    )  # [B, 3*DIM]
    bq_ext = np.ascontiguousarray(
        np.concatenate([bq_s, bias_total[:, None, :DIM]], axis=1)
    )  # [B, 33, DIM]
    bv_ext = np.ascontiguousarray(
        np.concatenate([bv_s, bias_total[:, None, 2 * DIM :]], axis=1)
    )  # [B, 33, DIM]
    # k-part bias replicated across the 128 token partitions
    bk_row = np.ascontiguousarray(
        np.broadcast_to(bias_total[:, None, DIM : 2 * DIM], (BSZ, 128, DIM))
    )  # [B, 128, DIM]
    ones = np.ones((2, SEQ), dtype=np.float32)

    nc = _get_nc()
    in_maps = []
    for c in range(NCORES):
        sl = slice(c * SPC, (c + 1) * SPC)
        in_maps.append(
            {
                "xT": xT[sl],
                "wT": wT,
                "acat": acat[sl],
                "bq": bq_ext[sl],
                "bv": bv_ext[sl],
                "bk": bk_row[sl],
                "ones": ones,
            }
        )
    global _LAST_IN_MAPS
    _LAST_IN_MAPS = in_maps
    res = run_bass_kernel_spmd(nc, in_maps, list(range(NCORES)))
    out = np.concatenate(
        [np.asarray(res.results[i]["out"]) for i in range(NCORES)], axis=0
    )
    return out.astype(np.float32, copy=False)


_LAST_IN_MAPS = None


# revision 37
# speedup vs baseline: 1.2226x; 1.0210x over previous
"""Trainium2 Bass kernel for MoE-LoRA fused QKV projection.

Computes, for x[32,512,1024], weight[3072,1024], per-sample top-2 LoRA
expert pools (rank 16) and expert biases:

    qkv = x @ W.T + bias
    qkv[..., :1024]  += SCALE * sum_k attn[b,k] * (x @ A_q[idx]) @ B_q[idx]
    qkv[..., 2048:]  += SCALE * sum_k attn[b,k] * (x @ A_v[idx]) @ B_v[idx]
    qkv += SCALE * sum_k attn[b,k] * bias_pool[idx]

Strategy: data-parallel over batch on 8 NeuronCores (4 samples/core).
Host side: transpose x / weight, gather + scale the tiny LoRA pools per
sample, fold all bias terms into one per-sample row vector that rides the
LoRA matmul via an appended ones-row (rank-1 matmul trick). Device side:
everything is dense fp32r (TF32-like, full PE rate) matmuls accumulated
in PSUM.
"""

import sys

for _p in ("/opt/trn_rl_repo",):
    if _p not in sys.path:
        sys.path.append(_p)

from contextlib import ExitStack

import numpy as np

import concourse.bass as bass
import concourse.tile as tile
from concourse import bacc, mybir
from concourse.bass_utils import run_bass_kernel_spmd

DIM = 1024
RANK = 16
TOPK = 2
BSZ = 32
SEQ = 512
SCALE = 1.0
NCORES = 8
SPC = BSZ // NCORES  # samples per core
NT = SEQ // 128  # token tiles per sample
KT = DIM // 128  # contraction tiles
NC3 = 3 * DIM
NCH = NC3 // 512  # output column chunks

F32 = mybir.dt.float32
F32R = mybir.dt.float32r

_NC_CACHE = {}


def _build_nc():
    nc = bacc.Bacc("TRN2", target_bir_lowering=False, debug=False)
    # all large inputs are pre-packed host-side into exact SBUF layouts so
    # every DMA is 2D-contiguous with multi-KB descriptors
    xT = nc.dram_tensor("xT", [SPC, 128, KT * SEQ], F32R, kind="ExternalInput").ap()
    wT = nc.dram_tensor("wT", [128, 3 * KT * 1024], F32R, kind="ExternalInput").ap()
    acat = nc.dram_tensor(
        "acat", [SPC, 128, KT * 4 * RANK], F32R, kind="ExternalInput"
    ).ap()
    bq = nc.dram_tensor("bq", [SPC, 33, DIM], F32R, kind="ExternalInput").ap()
    bv = nc.dram_tensor("bv", [SPC, 33, DIM], F32R, kind="ExternalInput").ap()
    bk = nc.dram_tensor("bk", [SPC, 128, DIM], F32, kind="ExternalInput").ap()
    ones = nc.dram_tensor("ones", [2, SEQ], F32R, kind="ExternalInput").ap()
    out = nc.dram_tensor("out", [SPC, SEQ, NC3], F32, kind="ExternalOutput").ap()

    with tile.TileContext(nc) as tc, ExitStack() as ctx:
        wpool = ctx.enter_context(tc.tile_pool(name="w", bufs=1))
        xpool = ctx.enter_context(tc.tile_pool(name="x", bufs=2))
        apool = ctx.enter_context(tc.tile_pool(name="a", bufs=2))
        bpool = ctx.enter_context(tc.tile_pool(name="b", bufs=2))
        tpool = ctx.enter_context(tc.tile_pool(name="t", bufs=2))
        opool = ctx.enter_context(tc.tile_pool(name="o", bufs=3))
        pst = ctx.enter_context(tc.tile_pool(name="pst", bufs=1, space="PSUM"))
        pso = ctx.enter_context(tc.tile_pool(name="pso", bufs=5, space="PSUM"))

        def load_sample(s):
            # A_cat first (tiny, unblocks the first t-matmul earliest)
            a_t = apool.tile([128, KT * 4 * RANK], F32R, tag="a")
            nc.sync.dma_start(a_t[:], acat[s])
            # x in two 1 MB halves so the first t-matmuls start early
            x_t = xpool.tile([128, KT * SEQ], F32R, tag="x")
            hx = KT * SEQ // 2
            for i in range(2):
                nc.sync.dma_start(
                    x_t[:, i * hx : (i + 1) * hx], xT[s, :, i * hx : (i + 1) * hx]
                )
            bq_sb = bpool.tile([33, DIM], F32R, tag="bq")
            nc.scalar.dma_start(bq_sb[:], bq[s])
            bv_sb = bpool.tile([33, DIM], F32R, tag="bv")
            nc.scalar.dma_start(bv_sb[:], bv[s])
            # k-part bias pre-broadcast across all 128 partitions: added
            # during PSUM evacuation (saves a PE matmul per k-chunk group)
            bk_sb = bpool.tile([128, DIM], F32, tag="bk")
            nc.scalar.dma_start(bk_sb[:], bk[s])
            return x_t, a_t, bq_sb, bv_sb, bk_sb

        loaded = {0: load_sample(0)}

        # Resident base weight: three 4 MB chunk-pair DMAs into one tile;
        # free layout (cp, k, col): w for (k, cp) at cp*8*1024 + k*1024
        w_all = wpool.tile([128, 3 * KT * 1024], F32R, tag="wall")
        wq = KT * 1024
        for h2 in range(2 * (NCH // 2)):
            nc.sync.dma_start(
                w_all[:, h2 * wq // 2 : (h2 + 1) * wq // 2],
                wT[:, h2 * wq // 2 : (h2 + 1) * wq // 2],
            )

        def wtile(k, cp):
            off = cp * wq + k * 1024
            return w_all[:, off : off + 1024]

        for s in range(SPC):
            if s not in loaded:
                loaded[s] = load_sample(s)
            x_t, a_t, bq_sb, bv_sb, bk_sb = loaded.pop(s)

            def xk(k):
                return x_t[:, bass.ts(k, SEQ)]

            # t_q[32, 512] = A_q_cat.T @ x ; t_v likewise (separate PSUM
            # tiles: a 33-partition matmul operand must start at base 0,
            # and copies must stay partition-aligned)
            pt_q = pst.tile([32, SEQ], F32, tag="ptq")
            pt_v = pst.tile([32, SEQ], F32, tag="ptv")
            for k in range(KT):
                nc.tensor.matmul(
                    pt_q[:],
                    a_t[:, k * 64 : k * 64 + 32],
                    xk(k),
                    start=(k == 0),
                    stop=(k == KT - 1),
                )
            for k in range(KT):
                nc.tensor.matmul(
                    pt_v[:],
                    a_t[:, k * 64 + 32 : (k + 1) * 64],
                    xk(k),
                    start=(k == 0),
                    stop=(k == KT - 1),
                )
            # tq/tv rows 0-31 = t, row 32 = ones (bias rides rank-1 row)
            tq_sb = tpool.tile([33, SEQ], F32R, tag="tq")
            tv_sb = tpool.tile([33, SEQ], F32R, tag="tv")
            nc.scalar.dma_start(tq_sb[32:33, :], ones[0:1, :])
            nc.scalar.dma_start(tv_sb[32:33, :], ones[0:1, :])
            nc.vector.tensor_copy(tq_sb[0:32, :], pt_q[:])
            nc.vector.tensor_copy(tv_sb[0:32, :], pt_v[:])

            # chunk-pair-outer so the base-weight stream is consumed at
            # DMA pace; stores batched per [128, 1024] chunk-pair.
            # Groups are emitted with the NEXT group's opening matmul
            # injected before this group's last two, so the fresh-PSUM-bank
            # issue penalty overlaps the previous group's streaming.
            groups = []
            for cp in range(NCH // 2):
                for nt in range(NT):
                    for half in range(2):
                        groups.append((cp, nt, half))

            o_tiles = {}

            def ensure_o(cp, nt):
                if (cp, nt) not in o_tiles:
                    o_tiles[(cp, nt)] = opool.tile([128, 1024], F32, tag="o", name="o")
                return o_tiles[(cp, nt)]

            def head_mm(gi, po):
                cp, nt, half = groups[gi]
                c = cp * 2 + half
                tok = bass.ts(nt, 128)
                if c < 2:
                    nc.tensor.matmul(
                        po[:], tq_sb[:, tok], bq_sb[:, bass.ts(c, 512)],
                        start=True, stop=False, skip_group_check=True,
                    )
                    return 0
                if c >= 4:
                    nc.tensor.matmul(
                        po[:], tv_sb[:, tok], bv_sb[:, bass.ts(c - 4, 512)],
                        start=True, stop=False, skip_group_check=True,
                    )
                    return 0
                nc.tensor.matmul(
                    po[:],
                    x_t[:, nt * 128 : nt * 128 + 128],
                    wtile(0, cp)[:, bass.ts(half, 512)],
                    start=True, stop=False, skip_group_check=True,
                )
                return 1  # consumed base k=0

            def base_mm(gi, po, k):
                cp, nt, half = groups[gi]
                nc.tensor.matmul(
                    po[:],
                    x_t[:, k * SEQ + nt * 128 : k * SEQ + (nt + 1) * 128],
                    wtile(k, cp)[:, bass.ts(half, 512)],
                    start=False, stop=(k == KT - 1), skip_group_check=True,
                )

            def evac(gi, po):
                cp, nt, half = groups[gi]
                c = cp * 2 + half
                o_sb = ensure_o(cp, nt)
                if 2 <= c < 4:
                    # k-part: bias added during evacuation (DVE)
                    nc.vector.tensor_add(
                        o_sb[:, bass.ts(half, 512)], po[:],
                        bk_sb[:, bass.ts(c - 2, 512)],
                    )
                else:
                    # split evacuation across DVE and ACT to shorten the
                    # PSUM-read window
                    nc.vector.tensor_copy(
                        o_sb[:, half * 512 : half * 512 + 256], po[:, 0:256]
                    )
                    nc.scalar.copy(
                        o_sb[:, half * 512 + 256 : half * 512 + 512],
                        po[:, 256:512],
                    )
                if half == 1:
                    nc.scalar.dma_start(
                        out[s, bass.ts(nt, 128), bass.ts(cp, 1024)], o_sb[:]
                    )
                    del o_tiles[(cp, nt)]

            po_of = {}
            kstart = {}
            po_of[0] = pso.tile([128, 512], F32, tag="po", name="po")
            kstart[0] = head_mm(0, po_of[0])
            for gi in range(len(groups)):
                ks = list(range(kstart[gi], KT))
                for j, k in enumerate(ks):
                    if j == len(ks) - 2 and gi + 1 < len(groups):
                        po_of[gi + 1] = pso.tile([128, 512], F32, tag="po", name="po")
                        kstart[gi + 1] = head_mm(gi + 1, po_of[gi + 1])
                    base_mm(gi, po_of[gi], k)
                evac(gi, po_of.pop(gi))

    nc.compile()
    return nc


def _get_nc():
    if "nc" not in _NC_CACHE:
        _NC_CACHE["nc"] = _build_nc()
    return _NC_CACHE["nc"]


def kernel(**inputs):
    x = np.asarray(inputs["x"], dtype=np.float32)
    weight = np.asarray(inputs["weight"], dtype=np.float32)
    bias = np.asarray(inputs["bias"], dtype=np.float32)
    A_q = np.asarray(inputs["A_q_pool"], dtype=np.float32)
    B_q = np.asarray(inputs["B_q_pool"], dtype=np.float32)
    A_v = np.asarray(inputs["A_v_pool"], dtype=np.float32)
    B_v = np.asarray(inputs["B_v_pool"], dtype=np.float32)
    bias_pool = np.asarray(inputs["bias_pool"], dtype=np.float32)
    attn = np.asarray(inputs["attn"], dtype=np.float32)
    idx = np.asarray(inputs["idx"]).astype(np.int64)

    # -- host-side prep: pack the big operands into exact SBUF layouts --
    # x[b, n, k*128+p] -> xT[b, p, k*SEQ + n]
    xT = np.ascontiguousarray(
        x.reshape(BSZ, SEQ, KT, 128).transpose(0, 3, 2, 1).reshape(BSZ, 128, KT * SEQ)
    )
    # weight.T[k*128+p, cp*1024+col] -> wT[p, cp*8192 + k*1024 + col]
    wT = np.ascontiguousarray(
        weight.T.reshape(KT, 128, 3, 1024)
        .transpose(1, 2, 0, 3)
        .reshape(128, 3 * KT * 1024)
    )

    i0, i1 = idx[:, 0], idx[:, 1]
    # [B, DIM, 64]: columns 0-15 q/k0, 16-31 q/k1, 32-47 v/k0, 48-63 v/k1
    acat = np.concatenate([A_q[i0], A_q[i1], A_v[i0], A_v[i1]], axis=2)
    # acat[b, k*128+p, r] -> [b, p, k*64 + r]
    acat = np.ascontiguousarray(
        acat.reshape(BSZ, KT, 128, 4 * RANK)
        .transpose(0, 2, 1, 3)
        .reshape(BSZ, 128, KT * 4 * RANK)
    )

    wgt = (SCALE * attn)[:, :, None, None]  # [B, K, 1, 1]
    bq_s = (B_q[idx] * wgt).reshape(BSZ, TOPK * RANK, DIM)
    bv_s = (B_v[idx] * wgt).reshape(BSZ, TOPK * RANK, DIM)
    bias_total = bias[None, :] + SCALE * np.einsum(
        "bko,bk->bo", bias_pool[idx], attn
    )  # [B, 3*DIM]
    bq_ext = np.ascontiguousarray(
        np.concatenate([bq_s, bias_total[:, None, :DIM]], axis=1)
    )  # [B, 33, DIM]
    bv_ext = np.ascontiguousarray(
        np.concatenate([bv_s, bias_total[:, None, 2 * DIM :]], axis=1)
    )  # [B, 33, DIM]
    # k-part bias replicated across the 128 token partitions
    bk_row = np.ascontiguousarray(
        np.broadcast_to(bias_total[:, None, DIM : 2 * DIM], (BSZ, 128, DIM))
    )  # [B, 128, DIM]
    ones = np.ones((2, SEQ), dtype=np.float32)

    nc = _get_nc()
    in_maps = []
    for c in range(NCORES):
        sl = slice(c * SPC, (c + 1) * SPC)
        in_maps.append(
            {
                "xT": xT[sl],
                "wT": wT,
                "acat": acat[sl],
                "bq": bq_ext[sl],
                "bv": bv_ext[sl],
                "bk": bk_row[sl],
                "ones": ones,
            }
        )
    global _LAST_IN_MAPS
    _LAST_IN_MAPS = in_maps
    res = run_bass_kernel_spmd(nc, in_maps, list(range(NCORES)))
    out = np.concatenate(
        [np.asarray(res.results[i]["out"]) for i in range(NCORES)], axis=0
    )
    return out.astype(np.float32, copy=False)


_LAST_IN_MAPS = None
